# revision 19
# baseline (speedup 1.0000x reference)
"""Trainium2 Bass kernel for nn_CrossModeAttention (B=4, N=1024, D1=D2=512,
C=512, H=8, DH=64, 3 cross-attention layers sharing one softmax matrix).

Each NeuronCore processes one full batch element (cores 4-7 duplicate 0-3).

Structure:
  phase 1+2 (unchanged from prior version): fp8 DoubleRow projections and
  QK^T; softmax exp split across ACT (native Exp->fp8) and DVE (Schraudolph
  int8-bitcast affine); V projection in bf16.

  phase 3 (rewritten): the old AV used P-stationary matmuls with a 65-wide
  free dim - 768 matmuls each paying a ~256-cycle LDWEIGHTS for ~32 cycles
  of work, leaving the PE under-occupied so HAM clock-throttled it to
  1.2 GHz for the last ~200us of the kernel.  Now V (fp8, +ones column) is
  the stationary and P [keys, queries] is the moving operand: 512-wide
  DoubleRow matmuls (256 cycles each) accumulate V1^T [dh, q] per head in
  PSUM; a bf16 SBUF copy + PE identity-transpose brings V1 back to [q, c]
  layout, where the LayerNorm pipeline (rowsum normalize, residual,
  bn_stats, normalize) runs as before but in bf16 to halve DVE cost.
  Layer 0 keeps the ones column through the transpose to extract the
  softmax row-sums (rs); layers 1-2 reuse rs and drop the extra column.
"""

import numpy as np

import concourse.bass as bass
from concourse import mybir
from concourse.bass_utils import run_bass_kernel_spmd
from concourse.tile import TileContext

B, N, D, C, H, DH = 4, 1024, 512, 512, 8, 64
NB = MB = N // 128          # 8 token/key blocks
KD = D // 128               # 4 contraction tiles per 512
LAYERS = 3
SCALE = float(D) ** -0.5
WSCALE = 32.0               # fp8 weight pre-scale for wq/wk
EXP_SCALE = SCALE / (WSCALE * WSCALE)
LN_EPS = 1e-5
SCH_A = 8.0 / np.log(2.0)   # schraudolph fp8e4m3 multiplier
SCH_B = 56.0                # 7 (e4m3 bias) * 8 ; calibrate +-0.5 for rounding

F32 = mybir.dt.float32
BF16 = mybir.dt.bfloat16
FP8 = mybir.dt.float8e4
I8 = mybir.dt.int8
AF = mybir.ActivationFunctionType
ALU = mybir.AluOpType
DR = mybir.MatmulPerfMode.DoubleRow


def _split_sync_waits(nc: bass.Bass) -> None:
    """Walrus codegen for this target accepts at most ONE sync wait per
    instruction, but Tile's scheduler may attach several (one per producer
    engine/DMA-queue clock). Insert single-wait NOPs on the same engine
    immediately before any multi-wait instruction — per-engine program
    order makes this semantically identical."""
    k = 0
    for f in nc.m.functions:
        for bb in f.blocks:
            newl = []
            changed = False
            for inst in bb.instructions:
                si = inst.sync_info
                waits = list(si.on_wait) if si and si.on_wait else []
                if len(waits) > 1:
                    for w in waits[:-1]:
                        nop = mybir.InstNoOp(name=f"WSPLIT-{k}", ins=[], outs=[])
                        k += 1
                        nop.engine = inst.engine
                        nop.sync_info = mybir.SyncInfo(on_wait=[w], on_update=[])
                        newl.append(nop)
                    si.on_wait = waits[-1:]
                    changed = True
                newl.append(inst)
            if changed:
                bb.instructions = newl


def _bcast(ap: bass.AP, count: int) -> bass.AP:
    """Append a step-0 (broadcast) free dimension of `count` to an AP."""
    return bass.AP(tensor=ap.tensor, offset=ap.offset, ap=[*ap.ap, [0, count]])


class Balancer:
    """Greedy least-finish-time dispatch across ACT / DVE / GpSimd."""

    RATE = {"act": 1.2e9, "dve": 1.0e9, "gp": 0.35e9}    # cols/s (measured)
    OVH = {"act": 350e-9, "dve": 200e-9, "gp": 600e-9}   # per-op overhead s

    def __init__(self):
        self.load = {k: 0.0 for k in self.RATE}

    def pick(self, cols, allowed=("act", "dve", "gp")):
        def fin(k):
            return self.load[k] + cols / self.RATE[k] + self.OVH[k]
        e = min(allowed, key=fin)
        self.load[e] = fin(e)
        return e


def build_kernel(with_gamma_beta: bool, _cls=bass.Bass) -> bass.Bass:
    nc = _cls()

    x1t = nc.dram_tensor("x1t", [D, N], BF16, kind="ExternalInput")
    x2t = nc.dram_tensor("x2t", [D, N], BF16, kind="ExternalInput")
    x18 = nc.dram_tensor("x18", [D, N], FP8, kind="ExternalInput")
    x28 = nc.dram_tensor("x28", [D, N], FP8, kind="ExternalInput")
    wq8 = nc.dram_tensor("wq8", [D, C], FP8, kind="ExternalInput")
    wk8 = nc.dram_tensor("wk8", [D, C], FP8, kind="ExternalInput")
    wvt = nc.dram_tensor("wvt", [2 * D, C], BF16, kind="ExternalInput")
    identd = nc.dram_tensor("identd", [128, 128], BF16, kind="ExternalInput")
    if with_gamma_beta:
        gamma = nc.dram_tensor("gamma", [C], F32, kind="ExternalInput")
        beta = nc.dram_tensor("beta", [C], F32, kind="ExternalInput")
    out = nc.dram_tensor("out", [N, C], F32, kind="ExternalOutput")

    bal = Balancer()

    # GpSimd cannot access PSUM, so PSUM-source work is ACT/DVE only.
    def copy_to(dst, src, cols, allowed=("act", "dve")):
        e = bal.pick(cols, allowed=allowed)
        if e == "act":
            nc.scalar.copy(dst, src)
        elif e == "dve":
            nc.vector.tensor_copy(dst, src)
        else:
            nc.gpsimd.tensor_copy(dst, src)

    def exp_to(P_ap, pt, cols):
        e = bal.pick(cols, allowed=("act", "dve"))
        if e == "act":
            nc.scalar.activation(out=P_ap, in_=pt, func=AF.Exp, scale=EXP_SCALE)
        else:
            nc.vector.tensor_scalar(
                P_ap.bitcast(I8), pt, scalar1=SCH_A * EXP_SCALE, scalar2=SCH_B,
                op0=ALU.mult, op1=ALU.add,
            )

    def add_to(dst, a, b, cols, allowed=("dve", "gp")):
        e = bal.pick(cols, allowed=allowed)
        if e == "dve":
            nc.vector.tensor_add(dst, a, b)
        else:
            nc.gpsimd.tensor_add(dst, a, b)

    with TileContext(nc) as tc:
        with tc.tile_pool(name="persist", bufs=1) as persist, \
             tc.tile_pool(name="qk", bufs=1) as qkpool, \
             tc.tile_pool(name="pp", bufs=1) as ppool, \
             tc.tile_pool(name="lay", bufs=2) as lay, \
             tc.tile_pool(name="gbp", bufs=1) as gbp, \
             tc.tile_pool(name="warm", bufs=1, space="PSUM") as warm_ps, \
             tc.tile_pool(name="stat", bufs=8) as stat:
            Vfb = persist.tile([128, NB, C], BF16)     # residual V (bf16)
            rs = persist.tile([128, NB, H], F32)       # 1/rowsum (f32)
            rsb = persist.tile([128, NB, H], BF16)     # 1/rowsum (bf16)
            ident = persist.tile([128, 128], BF16)
            eps_t = persist.tile([128, 1], F32)
            nc.vector.memset(eps_t, LN_EPS)
            nc.sync.dma_start(out=ident, in_=identd[:, :])

            wtile = warm_ps.tile([128, 128], F32, name="wtile")

            def warm_spin(n):
                """`n` dependency-free matmuls on `ident`: execute
                immediately in PE program order, keeping the HAM activity
                window busy through engine-stall stretches so the PE array
                is not clock-gated to 1.2 GHz."""
                for _ in range(n):
                    nc.tensor.matmul(wtile, lhsT=ident, rhs=ident,
                                     start=True, stop=True,
                                     skip_group_check=True)

            # PE is otherwise idle until input DMAs land (~20us): spin warm
            warm_spin(140)

            # QT8/KT8: head h lives in tile ht=h//3 at partition base
            # (h%3)*32 (legal AP bases are 0/32/64 only); dims [ht, ktile, n]
            QT8 = qkpool.tile([128, 3, 2, N], FP8)
            KT8 = qkpool.tile([128, 3, 2, N], FP8)
            P = ppool.tile([128, H, MB, N], FP8)


            # 66 value-columns per head: 64 V + ones (x2 so the DR ktile
            # stride H*66 is 16B aligned; row 65 of V1^T is unused)
            vaug = lay.tile([128, MB, H, 66], FP8, tag="vaug")
            nc.vector.memset(vaug[:, :, :, 64:66], 1.0)
            if with_gamma_beta:
                gb = gbp.tile([128, C], F32)
                bb = gbp.tile([128, C], F32)
                g_ap = gamma[:]
                b_ap = beta[:]
                nc.sync.dma_start(
                    out=gb,
                    in_=bass.AP(tensor=g_ap.tensor, offset=0, ap=[[0, 128], *g_ap.ap]),
                )
                nc.sync.dma_start(
                    out=bb,
                    in_=bass.AP(tensor=b_ap.tensor, offset=0, ap=[[0, 128], *b_ap.ap]),
                )

            # ---------------- phase 1+2: projections + QK^T + exp ------------
            # (V projection moved to phase 3: it shares the AV psum pool,
            # freeing 2 banks here so the QK->exp pipeline gets 3 bufs.)
            x1sb = qkpool.tile([128, KD, N], BF16)
            x2sb = qkpool.tile([128, KD, N], BF16)
            wv = qkpool.tile([128, 2 * KD, C], BF16)
            with tc.tile_pool(name="xs", bufs=1) as xs, \
                 tc.tile_pool(name="psq", bufs=1, space="PSUM") as psq, \
                 tc.tile_pool(name="psa", bufs=3, space="PSUM") as psa:
                x18sb = xs.tile([128, KD, N], FP8)
                x28sb = xs.tile([128, KD, N], FP8)
                wq8sb = xs.tile([128, KD, C], FP8)
                wk8sb = xs.tile([128, KD, C], FP8)

                def dma3(eng, dst_tile, src, t0, t1, src_cols):
                    """dst_tile[:, t0:t1, :] <- src rows [t0*128,(t1)*128)."""
                    eng.dma_start(
                        out=dst_tile[:, t0:t1, :],
                        in_=bass.AP(
                            tensor=src, offset=t0 * 128 * src_cols,
                            ap=[[src_cols, 128], [128 * src_cols, t1 - t0],
                                [1, src_cols]],
                        ),
                    )

                # Q/K path first (sync queue), V path on scalar queue.
                # halves so the first proj matmuls start sooner
                dma3(nc.sync, x18sb, x18, 0, 2, N)
                dma3(nc.sync, wq8sb, wq8, 0, 2, C)
                dma3(nc.sync, x28sb, x28, 0, 2, N)
                dma3(nc.sync, wk8sb, wk8, 0, 2, C)
                dma3(nc.sync, x18sb, x18, 2, 4, N)
                dma3(nc.sync, wq8sb, wq8, 2, 4, C)
                dma3(nc.sync, x28sb, x28, 2, 4, N)
                dma3(nc.sync, wk8sb, wk8, 2, 4, C)
                dma3(nc.scalar, x1sb, x1t, 0, 2, N)
                dma3(nc.scalar, x1sb, x1t, 2, 4, N)
                dma3(nc.scalar, x2sb, x2t, 0, 2, N)
                dma3(nc.scalar, x2sb, x2t, 2, 4, N)
                dma3(nc.scalar, wv, wvt, 0, 4, C)
                dma3(nc.scalar, wv, wvt, 4, 8, C)

                def qkproj_units(ht):
                    """(d, src, nh) units for head-tile ht: 2 DR matmuls +
                    fp8 copy of the packed [nh*32, 512] projection chunk."""
                    nh_heads = 2 if ht == 2 else 3
                    sz = nh_heads * 32
                    for d in range(2):
                        off = _PACK_OFF[ht * 2 + d]
                        for (w_sb, x_sb, dst) in (
                            (wq8sb, x18sb, QT8), (wk8sb, x28sb, KT8),
                        ):
                            for nh in range(2):
                                def unit(d=d, off=off, sz=sz, w_sb=w_sb,
                                         x_sb=x_sb, dst=dst, nh=nh, ht=ht):
                                    ps = psq.tile([128, 512], F32, tag="psq")
                                    for i in range(2):
                                        nc.tensor.matmul(
                                            ps[0:sz, :],
                                            lhsT=w_sb[:, 2 * i:2 * i + 2,
                                                      off:off + sz],
                                            rhs=x_sb[:, 2 * i:2 * i + 2,
                                                     nh * 512:(nh + 1) * 512],
                                            start=(i == 0), stop=(i == 1),
                                            perf_mode=DR,
                                        )
                                    copy_to(
                                        dst[0:sz, ht, d, nh * 512:(nh + 1) * 512],
                                        ps[0:sz, :], 512,
                                    )
                                yield unit

                def qk_exp_units(ht):
                    """Units for heads in tile `ht`: 2 DR matmuls + exp.
                    Heads rotate within each mb so consecutive stationaries
                    land at different PE row bases."""
                    heads = range(ht * 3, min(ht * 3 + 3, H))
                    for mb in range(MB):
                        for h in heads:
                            def unit(h=h, mb=mb, ht=ht):
                                pb = (h % 3) * 32
                                pt = psa.tile([128, N], F32, tag="psa")
                                for nh in range(2):
                                    nc.tensor.matmul(
                                        pt[:, nh * 512:(nh + 1) * 512],
                                        lhsT=KT8[pb:pb + 32, ht, :,
                                                 mb * 128:(mb + 1) * 128],
                                        rhs=QT8[pb:pb + 32, ht, :,
                                                nh * 512:(nh + 1) * 512],
                                        start=True, stop=True,
                                        perf_mode=DR,
                                    )
                                exp_to(P[:, h, mb, :], pt, 1024)
                            yield unit

                def run_with_fill(groups, fill):
                    k = 0
                    groups = list(groups)
                    for gi, g in enumerate(groups):
                        g()
                        want = (gi + 1) * len(fill) // len(groups)
                        while k < want:
                            fill[k]()
                            k += 1

                for u in qkproj_units(0):
                    u()
                run_with_fill(qk_exp_units(0), list(qkproj_units(1)))
                run_with_fill(qk_exp_units(1), list(qkproj_units(2)))
                for u in qk_exp_units(2):
                    u()

            # ---------------- phase 3: three AV + LayerNorm layers -----------
            # AV: V-stationary DoubleRow matmuls, P [keys, q] moving (512
            # free), accumulating V1^T [dh(+1), q] per head over 4 key-tile
            # pairs; then bf16 SBUF staging + PE identity-transpose back to
            # [q, c]; LN pipeline in bf16.
            with tc.tile_pool(name="psv1", bufs=1, space="PSUM") as psv1, \
                 tc.tile_pool(name="pstr", bufs=2, space="PSUM") as pstr, \
                 tc.tile_pool(name="y2p", bufs=3) as y2p, \
                 tc.tile_pool(name="vts", bufs=2) as vtsp:
                vtk = [0]  # rotating psum tag counter (6 banks)

                def vt_tile():
                    k = vtk[0] % 5
                    vtk[0] += 1
                    return psv1.tile([128, 512], F32, tag=f"v1t{k}",
                                     name=f"v1t{k}")

                # ---- V projection (bf16) -> Vfb + vaug, feeding layer 0
                for nb in range(NB):
                    ps = vt_tile()
                    for t in range(2 * KD):
                        x_sb = x1sb if t < KD else x2sb
                        nc.tensor.matmul(
                            ps,
                            lhsT=x_sb[:, t % KD, nb * 128:(nb + 1) * 128],
                            rhs=wv[:, t, :],
                            start=(t == 0), stop=(t == 2 * KD - 1),
                        )
                    copy_to(Vfb[:, nb, :], ps, 512)
                    copy_to(
                        vaug[:, nb, :, 0:64],
                        ps.rearrange("p (h d) -> p h d", d=DH), 512,
                    )

                for layer in range(LAYERS):
                    first = layer == 0
                    last = layer == LAYERS - 1
                    if not last:
                        vaug_next = lay.tile([128, MB, H, 66], FP8, tag="vaug")
                        nc.vector.memset(vaug_next[:, :, :, 64:66], 1.0)

                    for qh in range(2):
                        qsl = slice(qh * 512, (qh + 1) * 512)
                        # ---- AV matmuls -> V1^T psum, + bf16 staging ----
                        # 8 single-head tiles (66 rows: 64 V + rowsum + pad;
                        # DR output must start at partition 0), two groups
                        # of 4 heads sharing the 4 psum tags.
                        vts_g = []
                        for g in range(2):
                            vt_ps = [vt_tile() for i in range(4)]
                            vsb = vtsp.tile([65, 4, 512], BF16,
                                            tag=f"vts{g}",
                                            name=f"vts{g}")
                            for mtp in range(4):
                                for hs in range(4):
                                    h = 4 * g + hs
                                    nc.tensor.matmul(
                                        vt_ps[hs][0:66, :],
                                        lhsT=vaug[:, 2 * mtp:2 * mtp + 2,
                                                  h, :],
                                        rhs=P[:, h, 2 * mtp:2 * mtp + 2,
                                              qsl],
                                        start=(mtp == 0), stop=(mtp == 3),
                                        perf_mode=DR,
                                    )
                            for hs in range(4):
                                copy_to(vsb[:, hs, :], vt_ps[hs][0:65, :],
                                        512)
                            vts_g.append(vsb)

                        # ---- transpose + LN, stage-major across the two
                        # block-pairs of this half (breadth-first emission
                        # keeps each engine's 8-deep FIFO filled with ops
                        # whose deps are already satisfied) ----
                        y2s, stats = [], []
                        for npl in range(2):        # stage A: TR + mult + add
                            np0 = qh * 4 + npl * 2
                            y2 = y2p.tile([128, 2, C], BF16, tag="y2")
                            y2s.append(y2)
                            for g in range(2):
                                tr = pstr.tile([128, 2, 4, 66], BF16,
                                               tag="tr0")
                                for jj in range(2):
                                    lqb = npl * 2 + jj
                                    for hs in range(4):
                                        nc.tensor.transpose(
                                            tr[:, jj, hs, 0:65],
                                            vts_g[g][:, hs,
                                                     lqb * 128:
                                                     (lqb + 1) * 128],
                                            ident[0:65, 0:65],
                                        )
                                if first:
                                    nc.vector.reciprocal(
                                        rs[:, np0:np0 + 2, 4 * g:4 * g + 4],
                                        tr[:, :, :, 64],
                                    )
                                    nc.vector.tensor_copy(
                                        rsb[:, np0:np0 + 2, 4 * g:4 * g + 4],
                                        rs[:, np0:np0 + 2, 4 * g:4 * g + 4],
                                    )
                                nc.vector.tensor_mul(
                                    y2[:, :, 256 * g:256 * (g + 1)]
                                    .rearrange("p j (h d) -> p j h d",
                                               d=DH),
                                    tr[:, :, :, 0:64],
                                    _bcast(rsb[:, np0:np0 + 2,
                                               4 * g:4 * g + 4], DH),
                                )
                            add_to(y2, y2, Vfb[:, np0:np0 + 2, :], 1024)
                        for npl in range(2):        # stage B: bn stats + sqrt
                            y2 = y2s[npl]
                            rstd2 = stat.tile([128, 2], F32, tag="rstd")
                            negm2 = stat.tile([128, 2], F32, tag="negm")
                            mvs = []
                            for j in range(2):
                                st = stat.tile([128, 6], F32, tag=f"st{j}")
                                mv = stat.tile([128, 2], F32, tag=f"mv{j}")
                                nc.vector.bn_stats(st, y2[:, j, :])
                                nc.vector.bn_aggr(mv, st)
                                mvs.append(mv)
                            stats.append((rstd2, negm2, mvs))
                            for j in range(2):
                                nc.scalar.activation(
                                    out=rstd2[:, j:j + 1],
                                    in_=mvs[j][:, 1:2],
                                    func=AF.Sqrt, bias=eps_t, scale=1.0,
                                )
                        for npl in range(2):        # stage C: recip + negm
                            rstd2, negm2, mvs = stats[npl]
                            nc.vector.reciprocal(rstd2, rstd2)
                            for j in range(2):
                                nc.vector.tensor_scalar(
                                    negm2[:, j:j + 1], mvs[j][:, 0:1],
                                    scalar1=rstd2[:, j:j + 1], scalar2=-1.0,
                                    op0=ALU.mult, op1=ALU.mult,
                                )
                        for npl in range(2):        # stage D: normalize + out
                            np0 = qh * 4 + npl * 2
                            y2 = y2s[npl]
                            rstd2, negm2, mvs = stats[npl]
                            for j in range(2):
                                nb = np0 + j
                                if last:
                                    dest = lay.tile([128, C], F32, tag="osb")
                                elif with_gamma_beta:
                                    dest = lay.tile([128, C], BF16, tag="tmp")
                                else:
                                    dest = Vfb[:, nb, :]
                                nc.scalar.activation(
                                    out=dest, in_=y2[:, j, :],
                                    func=AF.Identity,
                                    scale=rstd2[:, j:j + 1],
                                    bias=negm2[:, j:j + 1],
                                )
                                if with_gamma_beta:
                                    fin = (lay.tile([128, C], F32, tag="osb")
                                           if last else Vfb[:, nb, :])
                                    nc.vector.tensor_mul(dest, dest, gb)
                                    nc.gpsimd.tensor_add(fin, dest, bb)
                                    dest = fin
                                if last:
                                    eng = nc.sync if nb % 2 == 0 else nc.scalar
                                    eng.dma_start(
                                        out=out[nb * 128:(nb + 1) * 128, :],
                                        in_=dest,
                                    )
                                else:
                                    copy_to(
                                        vaug_next[:, nb, :, 0:64],
                                        dest.rearrange("p (h d) -> p h d",
                                                       d=DH),
                                        512, allowed=("dve", "gp"),
                                    )
                        if qh == 1 and not last:
                            # cover the LN tail of the last half: the next
                            # layer's AV waits on vaug_next pairs
                            warm_spin(14)
                    if not last:
                        vaug = vaug_next

    _split_sync_waits(nc)
    return nc


_NPBF16 = mybir.dt.np(BF16)
_NPFP8 = mybir.dt.np(FP8)

# packed column order for wq8/wk8: groups (ht, d) of the heads in tile ht
# (3,3,2 heads), each head contributing its dh-half d columns.
# orig c = (3*ht + slot)*64 + d*32 + j
_PACK = np.empty(C, np.int64)
_PACK_OFF = []
_pos = 0
for _ht in range(3):
    _nh = 2 if _ht == 2 else 3
    for _d in range(2):
        _PACK_OFF.append(_pos)
        for _sl in range(_nh):
            for _j in range(32):
                _PACK[_pos] = (3 * _ht + _sl) * 64 + _d * 32 + _j
                _pos += 1
assert _pos == C


def make_in_maps(x1, x2, Wq, Wk, Wv, with_gb=False, g=None, bt=None):
    x1 = np.asarray(x1, np.float32)
    x2 = np.asarray(x2, np.float32)
    wq = np.ascontiguousarray((np.asarray(Wq, np.float32) * WSCALE).T[:, _PACK])
    wk = np.ascontiguousarray((np.asarray(Wk, np.float32) * WSCALE).T[:, _PACK])
    wv = np.ascontiguousarray(np.asarray(Wv, np.float32).T)
    ident = np.eye(128, dtype=np.float32)

    in_maps = []
    for i in range(8):
        b = i % B
        x1T = np.ascontiguousarray(x1[b].T)
        x2T = np.ascontiguousarray(x2[b].T)
        m = {
            "x1t": x1T.astype(_NPBF16),
            "x2t": x2T.astype(_NPBF16),
            "x18": x1T.astype(_NPFP8),
            "x28": x2T.astype(_NPFP8),
            "wq8": wq.astype(_NPFP8),
            "wk8": wk.astype(_NPFP8),
            "wvt": wv.astype(_NPBF16),
            "identd": ident.astype(_NPBF16),
        }
        if with_gb:
            m["gamma"] = g
            m["beta"] = bt
        in_maps.append(m)
    return in_maps


def kernel(x1, x2, Wq, Wk, Wv, ln_gamma, ln_beta):
    g = np.asarray(ln_gamma, np.float32)
    bt = np.asarray(ln_beta, np.float32)
    with_gb = not (np.all(g == 1.0) and np.all(bt == 0.0))

    in_maps = make_in_maps(x1, x2, Wq, Wk, Wv, with_gb, g, bt)
    nc = build_kernel(with_gb)
    res = run_bass_kernel_spmd(nc, in_maps, list(range(8)))
    return np.stack([res.results[b]["out"] for b in range(B)]).astype(np.float32)
